# revision 18
# baseline (speedup 1.0000x reference)
"""Trainium2 Bass kernel for nn_ContrastiveLoss (B=512, ZI=16, T=8, D=128).

Strategy: data-parallel over img batch (64 bi per core), text replicated.

v2 design notes:
  - text is NOT normalized before the matmul: 1/|text_row| is constant per
    sim-row (partition), so it commutes with the max over i and is folded
    into the exp as a per-partition scale AP (free on ScalarE).
  - img shard rows are host-reordered i-major (row = i*64 + j) so the max
    over i becomes a max over contiguous 512/256/128/64-wide column blocks
    (cheap tensor_tensor max tree) or a strided tensor_reduce.
  - PSUM evacuation + max is routed per q-tile across three engines:
    DVE direct reduce_max from PSUM, or ScalarE exp->SBUF(bf16) followed by
    a TT-max tree on DVE or GpSimd (exp commutes with max).
  - S_diag is recovered as sum(log(E_diag)) via a mask multiply (masks and
    the own-column mask are per-core host inputs).
  - one 16.9KB AllGather + local reduce combines den_t2i + scalar partials.
"""
import os
import numpy as np
import ml_dtypes

B, ZI, T, D = 512, 16, 8, 128
NC = 8
BL = B // NC            # 64 local bi
MLOC = BL * ZI          # 1024 img rows per core
NT = B * T              # 4096 text rows
PT = NT // 128          # 32 text partition-tiles (q)
NG = 4                  # groups of 8 q-tiles
QPG = PT // NG          # 8
DIAG_COEF = -(1.0 + 1.0 / T)

# per-q evacuation route: 'dve' = direct reduce_max from PSUM on VectorE;
# 'act_dve' = exp on ScalarE then TT-max tree on VectorE;
# 'act_gp' = exp on ScalarE then TT-max tree on GpSimd.
_ROUTE_PATTERN = ['dve', 'act_dve', 'act_dve', 'act_dve']


def _route(q):
    return _ROUTE_PATTERN[q % len(_ROUTE_PATTERN)]


_CACHE = {}


def _build_program():
    import concourse.bacc as bacc
    import concourse.mybir as mybir
    import concourse.tile as tile

    f32 = mybir.dt.float32
    bf16 = mybir.dt.bfloat16

    nc = bacc.Bacc("TRN2", num_devices=NC)
    img = nc.declare_dram_parameter("img", [MLOC, D], f32, isOutput=False)
    text = nc.declare_dram_parameter("text", [NT, D], f32, isOutput=False)
    masks = nc.declare_dram_parameter("masks", [128, PT * BL], bf16,
                                      isOutput=False)
    omc = nc.declare_dram_parameter("omc", [128, PT], f32, isOutput=False)
    ident = nc.declare_dram_parameter("ident", [128, 128], bf16,
                                      isOutput=False)
    out = nc.declare_dram_parameter("out", [1, 1], f32, isOutput=True)

    X = mybir.AxisListType.X
    MUL = mybir.AluOpType.mult
    ADD = mybir.AluOpType.add
    MAX = mybir.AluOpType.max
    EXP = mybir.ActivationFunctionType.Exp
    LN = mybir.ActivationFunctionType.Ln
    COPY = mybir.ActivationFunctionType.Copy

    with tile.TileContext(nc) as tc:
        with (
            tc.tile_pool(name="const", bufs=1) as cp,
            tc.tile_pool(name="sb", bufs=2) as sb,
            tc.tile_pool(name="raws", bufs=3) as rp,
            tc.tile_pool(name="eun", bufs=3) as ep,
            tc.tile_pool(name="ptp", bufs=1, space="PSUM") as ptp,
            tc.tile_pool(name="pmm", bufs=3, space="PSUM") as pmm,
            tc.tile_pool(name="psmall", bufs=1, space="PSUM") as pps,
            tc.tile_pool(name="dram", bufs=1, space="DRAM") as dp,
        ):
            ident_sb = cp.tile([128, 128], bf16)
            nc.sync.dma_start(ident_sb[:], ident[:])
            ones_bf = cp.tile([128, 1], bf16)
            nc.vector.memset(ones_bf[:], 1.0)
            ones_f = cp.tile([128, 1], f32)
            nc.vector.memset(ones_f[:], 1.0)

            tn_T = cp.tile([128, NT], bf16)    # raw text (bf16), [d, rows]
            im_T = cp.tile([128, MLOC], bf16)  # normalized img, [d, i*64+j]
            n2a = cp.tile([128, 40], f32)      # norms^2: img 0:8, text 8:40
            inva = cp.tile([128, 40], f32)     # rsqrt(n2a) via Newton
            den_a = cp.tile([128, 16], f32)    # den_t2i cols 0:16 (g0,g1)
            den_b = cp.tile([128, 16], f32)    # den_t2i cols 16:32 (g2,g3)
            em = cp.tile([128, PT], f32)       # masked E_diag per (q,p)

            SQUARE = mybir.ActivationFunctionType.Square

            # ---- img first: load, norms, normalize+cast, transpose ----
            def load_norm(src, base, s, n2_ap, sq_act=False):
                raw = rp.tile([128, 4, 128], f32, tag="raw", name=f"raw{s}")
                nc.sync.dma_start(
                    raw[:],
                    src[base:base + 512, :].rearrange("(k p) d -> p k d",
                                                      p=128),
                )
                sq = sb.tile([128, 4, 128], f32, tag="nsq", name=f"nsq{s}")
                if sq_act:
                    nc.scalar.activation(
                        sq[:].rearrange("p k d -> p (k d)"),
                        raw[:].rearrange("p k d -> p (k d)"), SQUARE)
                else:
                    nc.vector.tensor_tensor(sq[:], raw[:], raw[:], op=MUL)
                nc.vector.reduce_sum(n2_ap, sq[:], axis=X)
                return raw

            rawi = [load_norm(img, 512 * s, s, n2a[:, 4 * s:4 * s + 4])
                    for s in range(2)]

            def newton_rsqrt(inv_ap, n2_ap, w, tagp):
                # y0 = 11.3137/n2 ; 4x  y <- y*(1.5 - 0.5*n2*y^2)
                a = sb.tile([128, w], f32, tag=f"nw{tagp}a", name=f"nwa{tagp}")
                nc.vector.reciprocal(a[:], n2_ap)
                nc.vector.tensor_scalar(out=inv_ap, in0=a[:],
                                        scalar1=11.3137085, scalar2=None,
                                        op0=MUL)
                t = sb.tile([128, w], f32, tag=f"nw{tagp}t", name=f"nwt{tagp}")
                for _ in range(4):
                    nc.vector.tensor_tensor(t[:], inv_ap, inv_ap, op=MUL)
                    nc.vector.tensor_tensor(t[:], t[:], n2_ap, op=MUL)
                    nc.vector.tensor_scalar(out=t[:], in0=t[:],
                                            scalar1=-0.5, scalar2=1.5,
                                            op0=MUL, op1=ADD)
                    nc.vector.tensor_tensor(inv_ap, inv_ap, t[:], op=MUL)

            newton_rsqrt(inva[:, 0:8], n2a[:, 0:8], 8, "i")
            iinv = inva

            for s in range(2):
                nbi = sb.tile([128, 4, 128], bf16, tag="nbi", name=f"nbi{s}")
                for k in range(4):
                    nc.scalar.activation(
                        nbi[:, k, :], rawi[s][:, k, :], COPY,
                        scale=inva[:, 4 * s + k:4 * s + k + 1],
                    )
                tp = ptp.tile([128, 4, 128], bf16, tag="tp", name=f"tpi{s}")
                for k in range(4):
                    nc.tensor.transpose(tp[:, k, :], nbi[:, k, :], ident_sb[:])
                nc.scalar.copy(
                    im_T[:, 512 * s:512 * (s + 1)],
                    tp[:].rearrange("p k d -> p (k d)"),
                )

            # ---- text: load, norms (deferred scale), cast, transpose ----
            for s in range(8):
                raw = load_norm(text, 512 * s, 2 + s,
                                n2a[:, 8 + 4 * s:8 + 4 * s + 4], sq_act=True)
                nbt = sb.tile([128, 4, 128], bf16, tag="nbt", name=f"nbt{s}")
                nc.vector.tensor_copy(
                    nbt[:].rearrange("p k d -> p (k d)"),
                    raw[:].rearrange("p k d -> p (k d)"),
                )
                tp = ptp.tile([128, 4, 128], bf16, tag="tp", name=f"tpt{s}")
                for k in range(4):
                    nc.tensor.transpose(tp[:, k, :], nbt[:, k, :], ident_sb[:])
                nc.scalar.copy(
                    tn_T[:, 512 * s:512 * (s + 1)],
                    tp[:].rearrange("p k d -> p (k d)"),
                )
            lnt = sb.tile([128, PT], f32, tag="lnt", name="lnt")
            nc.scalar.activation(lnt[:], n2a[:, 8:40], LN)
            nc.scalar.activation(inva[:, 8:40], lnt[:], EXP, scale=-0.5)

            # ---- main loop ----
            masks_sb = cp.tile([128, PT * BL], bf16)
            nc.sync.dma_start(masks_sb[:], masks[:])
            omc_sb = cp.tile([128, PT], f32)
            nc.sync.dma_start(omc_sb[:], omc[:])

            ar1_in = dp.tile([128, 16], f32, name="ar1_in")
            ar1_out = dp.tile([NC * 128, 16], f32, addr_space="Shared",
                              name="ar1_out")
            ar2_in = dp.tile([128, 17], f32, name="ar2_in")
            ar2_out = dp.tile([NC * 128, 17], f32, addr_space="Shared",
                              name="ar2_out")
            dm_ps = pps.tile([1, 512], f32, tag="dmx", name="dm_ps")
            e_all = cp.tile([128, PT * BL], bf16)
            for g in range(NG):
                e_g = e_all[:, QPG * BL * g:QPG * BL * (g + 1)]
                for qr in range(QPG):
                    q = g * QPG + qr
                    ps = pmm.tile([128, 1024], f32, tag="ps", name=f"ps{q}")
                    for f in range(2):
                        nc.tensor.matmul(
                            ps[:, 512 * f:512 * (f + 1)],
                            lhsT=tn_T[:, 128 * q:128 * (q + 1)],
                            rhs=im_T[:, 512 * f:512 * (f + 1)],
                            start=True, stop=True,
                        )
                    ecols = e_all[:, QPG * BL * g + BL * qr:QPG * BL * g + BL * (qr + 1)]
                    r = _route(q)
                    if r == 'dve':
                        simq = sb.tile([128, BL], f32, tag="simq",
                                       name=f"sim{q}")
                        nc.vector.reduce_max(
                            simq[:],
                            ps[:].rearrange("p (i j) -> p j i", j=BL),
                            axis=X,
                        )
                        nc.scalar.activation(ecols, simq[:], EXP,
                                             scale=inva[:, 8 + q:9 + q])
                    else:
                        eun = ep.tile([128, 1024], bf16, tag="eun",
                                      name=f"eun{q}")
                        nc.scalar.activation(eun[:], ps[:], EXP,
                                             scale=inva[:, 8 + q:9 + q])
                        eng = nc.vector
                        t1 = ep.tile([128, 512], bf16, tag="t1",
                                     name=f"t1_{q}")
                        eng.tensor_tensor(t1[:], eun[:, 0:512],
                                          eun[:, 512:1024], op=MAX)
                        t2 = ep.tile([128, 256], bf16, tag="t2",
                                     name=f"t2_{q}")
                        eng.tensor_tensor(t2[:], t1[:, 0:256],
                                          t1[:, 256:512], op=MAX)
                        nc.vector.reduce_max(
                            ecols,
                            t2[:].rearrange("p (i j) -> p j i", j=BL),
                            axis=X,
                        )
                # den_t2i partial and masked diag for this group.
                # high_priority: the g0/g1 den reduces feed the overlapped
                # first AllReduce — without it the scheduler queues them at
                # the tail of the DVE stream and the collective can't start
                # until the main loop ends.
                den_dst = den_a if g < 2 else den_b
                with tc.high_priority():
                    nc.vector.reduce_sum(
                        den_dst[:, QPG * (g % 2):QPG * (g % 2 + 1)],
                        e_g.rearrange("p (q j) -> p q j", j=BL),
                        axis=X,
                    )
                nc.tensor.matmul(
                    dm_ps[:], lhsT=ones_bf[:], rhs=e_g,
                    start=(g == 0), stop=(g == NG - 1),
                    skip_group_check=True,
                )
                if g == 1:
                    with tc.high_priority():
                        nc.sync.dma_start(ar1_in[:], den_a[:])
                        nc.gpsimd.collective_compute(
                            "AllGather", mybir.AluOpType.bypass,
                            replica_groups=[list(range(NC))],
                            ins=[ar1_in[:].opt()],
                            outs=[ar1_out[:].opt()],
                        )

            # ---- masked diag extraction over the whole E ----
            scr2 = sb.tile([128, PT * BL], bf16, tag="scr2", name="scr2")
            nc.vector.tensor_tensor(scr2[:], e_all[:], masks_sb[:], op=MUL)
            nc.vector.reduce_sum(
                em[:],
                scr2[:].rearrange("p (q j) -> p q j", j=BL),
                axis=X,
            )

            # ---- local scalars ----
            den_i2t = sb.tile([1, BL], f32, tag="small", name="den_i2t")
            nc.vector.reduce_sum(
                den_i2t[:],
                dm_ps[0:1, :].rearrange("p (q j) -> p j q", q=QPG),
                axis=X,
            )
            lg = sb.tile([1, BL], f32, tag="small2", name="lg")
            la = sb.tile([1, 1], f32, tag="small3", name="la")
            nc.scalar.activation(lg[:], den_i2t[:], LN, accum_out=la[:])

            em2 = sb.tile([128, PT], f32, tag="em2", name="em2")
            nc.vector.tensor_tensor(em2[:], em[:], omc_sb[:], op=ADD)
            lem = sb.tile([128, PT], f32, tag="lem", name="lem")
            sd1 = sb.tile([128, 1], f32, tag="sd1", name="sd1")
            nc.scalar.activation(lem[:], em2[:], LN, accum_out=sd1[:])
            sd_ps = pps.tile([1, 1], f32, tag="dmx", name="sd_ps")
            nc.tensor.matmul(sd_ps[:], lhsT=ones_f[:], rhs=sd1[:],
                             start=True, stop=True)

            pt1 = sb.tile([1, 1], f32, tag="small5", name="pt1")
            nc.scalar.activation(pt1[:], sd_ps[:], COPY, scale=DIAG_COEF)
            part = sb.tile([1, 1], f32, tag="small6", name="part")
            nc.vector.tensor_tensor(part[:], la[:], pt1[:], op=ADD)

            colv = sb.tile([128, 1], f32, tag="small7", name="colv")
            nc.vector.memset(colv[:], 0.0)
            nc.vector.tensor_copy(colv[0:1, 0:1], part[:])

            # ---- second (tail) AllReduce: den_b + partial scalar ----
            nc.sync.dma_start(ar2_in[:, 0:16], den_b[:])
            nc.sync.dma_start(ar2_in[:, 16:17], colv[:])
            nc.gpsimd.collective_compute(
                "AllGather", mybir.AluOpType.bypass,
                replica_groups=[list(range(NC))],
                ins=[ar2_in[:].opt()],
                outs=[ar2_out[:].opt()],
            )
            arr1 = sb.tile([128, NC, 16], f32, tag="arr1", name="arr1")
            nc.sync.dma_start(
                arr1[:], ar1_out[:].rearrange("(r p) c -> p r c", p=128))
            arr2 = sb.tile([128, NC, 17], f32, tag="arr2", name="arr2")
            nc.sync.dma_start(
                arr2[:], ar2_out[:].rearrange("(r p) c -> p r c", p=128))
            arx1 = sb.tile([128, 16], f32, tag="arx1", name="arx1")
            nc.vector.reduce_sum(
                arx1[:], arr1[:].rearrange("p r c -> p c r"), axis=X)
            arx2 = sb.tile([128, 17], f32, tag="arx2", name="arx2")
            nc.vector.reduce_sum(
                arx2[:], arr2[:].rearrange("p r c -> p c r"), axis=X)

            lgt1 = sb.tile([128, 16], f32, tag="lgt1", name="lgt1")
            ls1 = sb.tile([128, 1], f32, tag="small8", name="ls1")
            nc.scalar.activation(lgt1[:], arx1[:], LN, accum_out=ls1[:])
            lgt2 = sb.tile([128, 16], f32, tag="lgt2", name="lgt2")
            ls2 = sb.tile([128, 1], f32, tag="smal28", name="ls2")
            nc.scalar.activation(lgt2[:], arx2[:, 0:16], LN, accum_out=ls2[:])
            lsb = sb.tile([128, 1], f32, tag="small9", name="lsb")
            nc.vector.tensor_tensor(lsb[:], ls1[:], ls2[:], op=ADD)
            fin_ps = pps.tile([1, 1], f32, tag="dmx", name="fin_ps")
            nc.tensor.matmul(fin_ps[:], lhsT=ones_f[:], rhs=lsb[:],
                             start=True, stop=True)
            res = sb.tile([1, 1], f32, tag="small10", name="res")
            nc.vector.tensor_tensor(res[:], fin_ps[:],
                                    arx2[0:1, 16:17], op=ADD)
            nc.sync.dma_start(out[:], res[:])

    nc.finalize()
    return nc


def _make_mask(c):
    m = np.zeros((128, PT * BL), np.float32)
    p = np.arange(128)
    for k in range(4):
        q = 4 * c + k
        j = 16 * k + p // 8
        m[p, q * BL + j] = 1.0
    return m.astype(ml_dtypes.bfloat16)


def _make_omc(c):
    """1 - colmask: 0 on this core's own 4 q-columns, 1 elsewhere."""
    m = np.ones((128, PT), np.float32)
    m[:, 4 * c:4 * c + 4] = 0.0
    return m


def _get_program():
    if "nc" not in _CACHE:
        _CACHE["nc"] = _build_program()
    return _CACHE["nc"]


def _install_trace_shim():
    """Register the NTFF profile hook that this container's antenv lacks.

    Only used by the local test harness (KERNEL_TRACE=1); the grading
    path never enters here.
    """
    import sys
    import types
    import antenv
    import concourse.bass_utils as bu
    from trn_agent_boot.trn_boot import _ntff_profile_via_ctypes

    if "antenv.axon_hooks" not in sys.modules:
        hook = _ntff_profile_via_ctypes("/opt/axon/libaxon_pjrt.so")
        mod = types.ModuleType("antenv.axon_hooks")
        mod.get_axon_ntff_profile_hook = lambda: hook
        mod.set_axon_ntff_profile_hook = lambda h: None
        sys.modules["antenv.axon_hooks"] = mod
        antenv.axon_hooks = mod
    bu.upload_artifacts = lambda tmpdir: tmpdir


def kernel(img: np.ndarray, text: np.ndarray) -> np.ndarray:
    from concourse.bass_utils import run_bass_kernel_spmd

    nc = _get_program()
    img = np.ascontiguousarray(np.asarray(img, dtype=np.float32))
    text = np.ascontiguousarray(np.asarray(text, dtype=np.float32))
    text_flat = text.reshape(NT, D)
    ident = np.eye(128, dtype=ml_dtypes.bfloat16)

    in_maps = []
    for c in range(NC):
        sh = img[BL * c:BL * (c + 1)].reshape(BL, ZI, D)
        # i-major row order: row = i*64 + j
        sh = np.ascontiguousarray(sh.transpose(1, 0, 2).reshape(MLOC, D))
        in_maps.append({
            "img": sh,
            "text": text_flat,
            "masks": _make_mask(c),
            "omc": _make_omc(c),
            "ident": ident,
        })

    trace = bool(int(os.environ.get("KERNEL_TRACE", "0")))
    if trace:
        _install_trace_shim()
    r = run_bass_kernel_spmd(nc, in_maps, core_ids=list(range(NC)),
                             trace=trace)
    _CACHE["last_result"] = r
    val = np.float32(r.results[0]["out"][0, 0])
    return np.asarray(val, dtype=np.float32).reshape(())
